# revision 108
# baseline (speedup 1.0000x reference)
"""AttentionReadout Trainium2 kernel.

Math (per graph g, N=96 padded rows, D=128 node dim, H=8 heads, HD=256):
  x_dense [96,128] (zero-padded), mask on QUERY rows only; keys/values keep
  padded rows (k_pad = bk, v_pad = bv).
  out_g = sum_n ( softmax_m(scale * q k^T)[n] @ v ) @ Wo + bo, summed over all
  96 dense rows (invalid query rows give uniform 1/96 attention).

Kernel algebra (what runs on device):
  - scores: S = X (scale Wq_h Wk_h^T) X^T + 1 bb^T, bb = X (scale Wk_h bq_h).
    Query-side bias terms are constant per row -> cancel in softmax.
  - M_h = scale*Wq_h@Wk_h^T and bb_h = scale*Wk_h@bq_h precomputed on host;
    bb folded into the PSUM->SBUF copies as a per-partition scalar/bias add
    (ACT Identity-with-bias / DVE tensor_scalar), replacing v1's rank-1
    bias matmuls on PE.
  - row weights: w_h[m] = sum_{n valid} E[n,m]/denom[n] + (96-size)/96
    (uniform correction for invalid query rows), E = exp(S).
  - denom[n] = sum_{m<kb} E[n,m] + (96-kb)  (keys >= kb have E==exp(0)==1).
  - z_{g,h} = X_g^T w_{g,h};  out_g = sum_h P_h^T z_{g,h} + co with
    P_h = Wv_h @ Wo_h precomputed on host (kills the Wv/Wo DMAs) and
    co = 96*(bv@Wo + bo) added via a rank-1 (corow x ones) PE matmul
    (v bias handled analytically: attention rows sum to 1).

Schedule (what made it fast under the TimelineSim cost model):
  - Heads run in pairs; each pair's exp and denominator reduce are merged
    2-head instructions (halves ACT/DVE fixed overheads).
  - Depth-2 software pipeline: pair p+1's Rt matmuls + PSUM->SBUF copies
    are emitted right after pair p's exps, so copies overlap the softmax.
  - B-bucket (small graphs, kb=kb_b) is packed 63-wide everywhere,
    including the xt input itself (636-col DMA instead of 768): tighter
    copies, 63-row exp/reduce, per-slot score matmuls.
  - Cold constants (co-row, ones) ride a second tiny SWDGE DMA so the
    hot msb/mask bundle lands earlier.
  - Engine placement under HW legality (GPSIMD cannot touch PSUM, no DVE
    divide): exp + q0-copies on ACT; reduces, recips, q1-copies, w+uc add
    on DVE; mask-multiplies and the denominator pad-add on Pool.
  - w-matmul blocks run at lag-1 through a pending queue so they never
    block the next pair's PE work.
  - A 22-matmul dummy chain warms the PE p-state during the input DMAs;
    the hot d1 bundle rides Pool SWDGE in parallel with the SP HWDGE queue.
  - PSUM: 4 banks rt (four 1-bank tiles/pair, bufs=4) + 2 sA + 1 sB +
    1 w/z/f.

Sharding: data-parallel, 8 graphs per core, 8 cores (sorted dealing:
slot j of core c holds graph order[j*8+c]; slots 0-3 big, 4-7 small).
"""

import sys

sys.path.insert(0, "/opt/trn_rl_repo")

import numpy as np
import ml_dtypes

import concourse.bass as bass
import concourse.bacc as bacc
import concourse.tile as tile
from concourse import mybir
from concourse import bass_utils

BF16 = mybir.dt.bfloat16
F32 = mybir.dt.float32
AF = mybir.ActivationFunctionType
ALU = mybir.AluOpType

B = 64
ND = 128          # node feature dim
HD = 256          # per-head hidden
H = 8             # heads
D = HD * H        # 2048
NP = 96           # padded rows per graph
NC = 8            # cores
G = B // NC       # graphs per core
SCALE = 1.0 / np.sqrt(np.float32(ND))

_CACHE = {}


def _build_program(kb_b=NP):
    """kb_b: key-column bound for slots 4-7 (the small-graph half after
    sorted dealing)."""
    nc = bacc.Bacc("TRN2", target_bir_lowering=False, debug=False,
                   num_devices=NC)

    GRP = 4                       # graphs per score bucket
    KA, KB_ = NP, kb_b            # key bounds per bucket
    WA, WB = GRP * KA, GRP * KB_  # per-head widths per bucket
    RTW = WA + WB                 # rt_sb: A 96-slots | B 63-packed
    AUX = 2 * ND + 40                # d1 hot cols

    # DRAM I/O (per-core shapes)
    # d1: msb01 | mkA 8 | mkB 8 | uc 16 | bb 8;  d2: corow | ones
    d1_d = nc.dram_tensor("d1", [ND, AUX], BF16, kind="ExternalInput").ap()
    d2_d = nc.dram_tensor("d2", [1, ND + G], BF16, kind="ExternalInput").ap()
    xt_d = nc.dram_tensor("xt", [ND, WA + WB], BF16,
                          kind="ExternalInput").ap()
    mr_d = nc.dram_tensor("mr", [ND, 6 * ND], BF16, kind="ExternalInput").ap()
    xr_d = nc.dram_tensor("xr", [NP, G * ND], BF16, kind="ExternalInput").ap()
    ph_d = nc.dram_tensor("ph", [ND, H * ND], BF16, kind="ExternalInput").ap()
    out_d = nc.dram_tensor("out", [ND, G], F32, kind="ExternalOutput").ap()

    with tile.TileContext(nc) as tc:
        with (
            tc.tile_pool(name="const", bufs=1) as cpool,
            tc.tile_pool(name="rt", bufs=4) as rtpool,
            tc.tile_pool(name="esb", bufs=3) as epool,
            tc.tile_pool(name="sm", bufs=4) as smpool,
            tc.tile_pool(name="acc", bufs=1) as apool,
            tc.tile_pool(name="rtp", bufs=4, space="PSUM") as rtp,
            tc.tile_pool(name="spa", bufs=1, space="PSUM") as spa,
            tc.tile_pool(name="spb", bufs=1, space="PSUM") as spb,
            tc.tile_pool(name="wz", bufs=1, space="PSUM") as wzp,
        ):
            # ---- input DMAs (d1 via Pool SWDGE overlaps SP HWDGE queue) ----
            d1 = cpool.tile([ND, AUX], BF16)
            nc.gpsimd.dma_start(d1[:], d1_d)
            d2 = cpool.tile([1, ND + G], BF16)
            nc.gpsimd.dma_start(d2[:], d2_d)
            xt = cpool.tile([ND, WA + WB], BF16)
            nc.sync.dma_start(xt[:, 0:WA], xt_d[:, 0:WA])
            nc.sync.dma_start(xt[:, WA:], xt_d[:, WA:])
            mr = cpool.tile([ND, 6 * ND], BF16)
            nc.sync.dma_start(mr[:], mr_d)
            xr = cpool.tile([NP, G * ND], BF16)
            nc.sync.dma_start(xr[:], xr_d)
            ph = cpool.tile([ND, H * ND], BF16)
            nc.sync.dma_start(ph[:], ph_d)

            # ---- warm-up: PE p-state ramp chain + ACT exp table prefetch ----
            zrow = cpool.tile([1, 128], BF16)
            nc.vector.memset(zrow[:], 0.0)
            lut0 = cpool.tile([1, 1], F32)
            nc.vector.memset(lut0[:], 0.0)
            lut1 = cpool.tile([1, 1], F32)
            nc.scalar.activation(lut1[:], lut0[:], AF.Exp)
            dmy = wzp.tile([1, 128], F32, tag="wz", name="dmy")
            for i in range(22):
                nc.tensor.matmul(dmy[:], zrow[:, 0:1], zrow[:],
                                 start=True, stop=True)

            msb01 = d1[:, 0:2 * ND]
            corow = d2[0:1, 0:ND]                     # co as a row
            ones8 = d2[0:1, ND:ND + G]
            # widen the aux block (mk, uc, bb) to f32 once on arrival
            auxf = cpool.tile([ND, 40], F32)
            nc.vector.tensor_copy(auxf[:], d1[:, 2 * ND:2 * ND + 40])
            mkA = auxf[0:NP, 0:8]                     # mask, big graphs
            mkB = auxf[0:KB_, 8:16]                   # mask, small graphs
            uc = auxf[0:NP, 16:32]                    # uniform corr
            bb = auxf[:, 32:40]                       # key-side bias, col h

            wt64 = apool.tile([NP, H * G], BF16)      # col h*G+g

            # ---- phase 1: 4 head-pairs, depth-2 software pipeline ----
            # rt PSUM: four 1-bank tiles per pair, pool bufs=3 so each
            # tile's WAR partner is an early-freed one
            def emit_rt_and_copies(p):
                sbs = []
                rtps = []
                for q in range(2):
                    h = 2 * p + q
                    msrc = msb01 if p == 0 else mr
                    mc = h * ND if p == 0 else (h - 2) * ND
                    rtA = rtp.tile([ND, WA], F32, tag="rtp",
                                   name=f"rtA{p}_{q}")
                    rtB = rtp.tile([ND, WB], F32, tag="rtp",
                                   name=f"rtB{p}_{q}")
                    nc.tensor.matmul(
                        rtA[:], msrc[:, mc:mc + ND],
                        xt[:, 0:WA], start=True, stop=True,
                    )
                    for gi in range(GRP):
                        nc.tensor.matmul(
                            rtB[:, gi * KB_:(gi + 1) * KB_],
                            msrc[:, mc:mc + ND],
                            xt[:, WA + gi * KB_:WA + (gi + 1) * KB_],
                            start=True, stop=True,
                        )
                    rtps.append((rtA, rtB))
                for q in range(2):
                    h = 2 * p + q
                    rtA, rtB = rtps[q]
                    rt_sb = rtpool.tile([ND, RTW], BF16, tag="rt",
                                        name=f"rtsb{p}_{q}")
                    # PSUM->SBUF + key-side bias bb_h (per-partition
                    # scalar). GPSIMD can't read PSUM: ACT/DVE only, split
                    # for balance: ACT gets cB-q0 + part of cA-q0.
                    if q == 0:
                        nc.scalar.activation(
                            rt_sb[:, 0:WA], rtA[:],
                            AF.Identity, bias=bb[:, h:h + 1])
                        nc.scalar.activation(
                            rt_sb[:, WA:RTW], rtB[:],
                            AF.Identity, bias=bb[:, h:h + 1])
                    else:
                        nc.vector.tensor_scalar_add(
                            rt_sb[:, 0:WA], rtA[:], bb[:, h:h + 1])
                        nc.vector.tensor_scalar_add(
                            rt_sb[:, WA:RTW], rtB[:], bb[:, h:h + 1])
                    sbs.append(rt_sb)
                return sbs

            def emit_w_block(p, eA, eB, rvA, rvB):
                # w_{h,g}[m] = sum_n E[n,m] rv[n]; w_ps cols (bkt, head, g)
                w_ps = wzp.tile([NP, 16], F32, tag="wz", name=f"w{p}")
                for q in range(2):
                    for g in range(GRP):
                        nc.tensor.matmul(
                            w_ps[0:KA, q * GRP + g:q * GRP + g + 1],
                            eA[:, q * WA + g * KA:q * WA + (g + 1) * KA],
                            rvA[:, q * GRP + g:q * GRP + g + 1],
                            start=True, stop=True,
                        )
                    for g in range(GRP):
                        nc.tensor.matmul(
                            w_ps[0:KB_, 8 + q * GRP + g:8 + q * GRP + g + 1],
                            eB[:, q * WB + g * KB_:q * WB + (g + 1) * KB_],
                            rvB[:, q * GRP + g:q * GRP + g + 1],
                            start=True, stop=True,
                        )
                # wt64[:, h*8+g] = w + uc  (w rows beyond kb are stale PSUM;
                # they multiply zero x-rows in the z matmul)
                h0 = 2 * p
                nc.vector.tensor_tensor(
                    wt64[:, h0 * G:(h0 + 2) * G].rearrange(
                        "p (q b g) -> p q b g", q=2, b=2),
                    w_ps[:].rearrange("p (b q g) -> p q b g", b=2, q=2),
                    uc[:].rearrange("p (b q g) -> p q b g", b=2, q=2),
                    op=ALU.add,
                )

            pend_w = []
            rts = emit_rt_and_copies(0)
            for p in range(4):
                sA = spa.tile([ND, 1024], F32, tag="sA")
                sB = spb.tile([ND, 512], F32, tag="sB")
                for q in range(2):
                    rt_sb = rts[q]
                    for g in range(GRP):
                        nc.tensor.matmul(
                            sA[0:NP, q * 512 + g * KA:q * 512 + (g + 1) * KA],
                            rt_sb[:, g * NP:(g + 1) * NP],
                            xt[:, g * NP:g * NP + KA],
                            start=True, stop=True,
                        )
                for q in range(2):
                    rt_sb = rts[q]
                    for g in range(GRP):
                        nc.tensor.matmul(
                            sB[0:KB_,
                               q * 256 + g * KB_:q * 256 + (g + 1) * KB_],
                            rt_sb[:, WA + g * KB_:WA + (g + 1) * KB_],
                            xt[:, WA + g * KB_:WA + (g + 1) * KB_],
                            start=True, stop=True,
                        )
                # merged exp over both heads of the pair
                eA = epool.tile([NP, 2 * WA], BF16, tag="eA")
                eB = epool.tile([KB_, 2 * WB], BF16, tag="eB")
                nc.scalar.activation(
                    eA[:].rearrange("p (b c) -> p b c", b=2),
                    sA[:].rearrange("p (b c) -> p b c", b=2)[0:NP, :, 0:WA],
                    AF.Exp,
                )
                nc.scalar.activation(
                    eB[:].rearrange("p (b c) -> p b c", b=2),
                    sB[:].rearrange("p (b c) -> p b c", b=2)[0:KB_, :, 0:WB],
                    AF.Exp,
                )
                rts = emit_rt_and_copies(p + 1) if p < 3 else None
                # merged denominator reduces, then rv = mask/denom per
                # bucket (DVE reciprocal + Pool mask-multiply, v1-style)
                dnA = smpool.tile([NP, 8], F32, tag="dnA")
                dnB = smpool.tile([KB_, 8], F32, tag="dnB")
                def red_A():
                    nc.vector.tensor_reduce(
                        dnA[:].rearrange("p (b q) -> p b q", b=2),
                        eA[:].rearrange("p (b q c) -> p b q c", b=2, c=KA),
                        op=ALU.add, axis=mybir.AxisListType.X,
                    )

                def red_B():
                    nc.vector.tensor_reduce(
                        dnB[:].rearrange("p (b q) -> p b q", b=2),
                        eB[:].rearrange("p (b q c) -> p b q c", b=2, c=KB_),
                        op=ALU.add, axis=mybir.AxisListType.X,
                    )

                red_A(); red_B()
                rcA = smpool.tile([NP, 8], F32, tag="rcA")
                nc.vector.reciprocal(rcA[:], dnA[:])
                rvA = smpool.tile([NP, 8], BF16, tag="rvA")
                nc.gpsimd.tensor_tensor(rvA[:], mkA[:], rcA[:], op=ALU.mult)
                dcB = smpool.tile([KB_, 8], F32, tag="dcB")
                nc.gpsimd.tensor_scalar_add(dcB[:], dnB[:], float(NP - kb_b))
                rcB = smpool.tile([KB_, 8], F32, tag="rcB")
                nc.vector.reciprocal(rcB[:], dcB[:])
                rvB = smpool.tile([KB_, 8], BF16, tag="rvB")
                nc.gpsimd.tensor_tensor(rvB[:], mkB[:], rcB[:], op=ALU.mult)
                pend_w.append((p, eA, eB, rvA, rvB))
                if len(pend_w) > 1:
                    emit_w_block(*pend_w.pop(0))
            for blk in pend_w:
                emit_w_block(*blk)

            # ---- phase 2: z_g = X_g^T @ wt (all heads at once) ----
            z_ps = wzp.tile([ND, G * H], F32, tag="wz", name="z")
            for g in range(G):
                nc.tensor.matmul(
                    z_ps[:, g * H:(g + 1) * H], xr[:, g * ND:(g + 1) * ND],
                    wt64[:, g::G], start=True, stop=True,
                )
            z64 = apool.tile([ND, G * H], BF16)       # col g*H+h
            nc.vector.tensor_copy(z64[:], z_ps[:])

            # ---- phase 3: out = sum_h P_h^T z_h + co (rank-1 matmul) ----
            f_ps = wzp.tile([ND, G], F32, tag="wz", name="f")
            for h in range(H):
                nc.tensor.matmul(
                    f_ps[:], ph[:, h * ND:(h + 1) * ND],
                    z64[:, h::H],
                    start=(h == 0), stop=False,
                )
            nc.tensor.matmul(f_ps[:], corow, ones8, start=False, stop=True)
            o_sb = smpool.tile([ND, G], F32, tag="osb", bufs=1)
            nc.vector.tensor_copy(o_sb[:], f_ps[:])
            nc.sync.dma_start(out_d, o_sb[:])

    nc.compile()
    return nc


def _prep_inputs(x, batch, Wq, bq, Wk, bk, Wv, bv, Wo, bo):
    x = np.asarray(x, np.float32)
    batch = np.asarray(batch, np.int64)
    counts = np.bincount(batch, minlength=B).astype(np.int64)
    starts = np.cumsum(counts) - counts
    # sorted dealing: slot j of core c processes graph order[j*NC+c], so
    # slots 4-7 hold the 32 smallest graphs -> key bound kb_b
    order = np.argsort(-counts, kind="stable")
    kb_b = int(counts[order[B // 2]])
    kb = [NP] * (G // 2) + [kb_b] * (G // 2)

    scale = np.float32(SCALE)
    # per-head fused score matrices and key-side bias vectors
    Wq3 = np.asarray(Wq, np.float32).reshape(ND, H, HD)
    Wk3 = np.asarray(Wk, np.float32).reshape(ND, H, HD)
    bq2 = np.asarray(bq, np.float32).reshape(H, HD)
    M = scale * np.einsum("chd,ehd->hce", Wq3, Wk3)          # [H,128,128]
    bbv = scale * np.einsum("chd,hd->hc", Wk3, bq2)          # [H,128]
    m_all = np.ascontiguousarray(
        M.transpose(1, 0, 2).reshape(ND, H * ND))            # [128, H*128]

    Wo_f = np.asarray(Wo, np.float32)
    Wv_f = np.asarray(Wv, np.float32)
    co = (NP * (np.asarray(bv, np.float32) @ Wo_f
                + np.asarray(bo, np.float32)))               # [128]
    # P_h = Wv_h @ Wo_h, laid out [e, (h, c)]
    P = np.einsum("ehd,hdc->hec",
                  Wv_f.reshape(ND, H, HD).transpose(0, 1, 2),
                  Wo_f.reshape(H, HD, ND))                   # [H,128,128]
    ph_host = np.ascontiguousarray(
        P.transpose(1, 0, 2).reshape(ND, H * ND)).astype(ml_dtypes.bfloat16)

    in_maps = []
    for c in range(NC):
        xt = np.zeros((ND, 4 * NP + 4 * kb_b), np.float32)
        xr = np.zeros((NP, G * ND), np.float32)
        d1 = np.zeros((ND, 2 * ND + 40), np.float32)
        d1[:, 0:2 * ND] = m_all[:, 0:2 * ND]
        d1[:, 2 * ND + 32:2 * ND + 40] = bbv.T
        d2 = np.zeros((1, ND + G), np.float32)
        d2[0, 0:ND] = co
        d2[0, ND:] = 1.0
        for j in range(G):
            g = int(order[j * NC + c])
            n = int(counts[g])
            xg = x[starts[g]:starts[g] + n]          # [n,128]
            if j < GRP_HOST:
                xt[:, j * NP:j * NP + n] = xg.T
            else:
                off = 4 * NP + (j - GRP_HOST) * kb_b
                xt[:, off:off + n] = xg.T
            xr[:n, j * ND:(j + 1) * ND] = xg
            # mask/uc cols (bkt, head, g): bucket = j//4, head-halves same
            bkt, gi = j // GRP_HOST, j % GRP_HOST
            for q in range(2):
                col = 2 * ND + bkt * 8 + q * GRP_HOST + gi
                d1[:n, col] = 1.0                               # mask
                d1[:NP, col + 16] = (NP - n) / np.float32(NP)   # uniform corr
        in_maps.append({
            "d1": d1.astype(ml_dtypes.bfloat16),
            "d2": d2.astype(ml_dtypes.bfloat16),
            "xt": xt.astype(ml_dtypes.bfloat16),
            "mr": m_all[:, 2 * ND:].astype(ml_dtypes.bfloat16),
            "xr": xr.astype(ml_dtypes.bfloat16),
            "ph": ph_host,
        })
    return in_maps, (order, kb_b)


GRP_HOST = 4


def kernel(x, batch, Wq, bq, Wk, bk, Wv, bv, Wo, bo, _trace=False):
    in_maps, (order, kb_b) = _prep_inputs(
        x, batch, Wq, bq, Wk, bk, Wv, bv, Wo, bo)
    key = ("nc", kb_b)
    if key not in _CACHE:
        _CACHE[key] = _build_program(kb_b)
    nc = _CACHE[key]
    res = bass_utils.run_bass_kernel_spmd(
        nc, in_maps, core_ids=list(range(NC)), trace=_trace,
    )
    _CACHE["last_result"] = res
    out = np.empty((B, ND), np.float32)
    for c in range(NC):
        o = np.asarray(res.results[c]["out"])     # [ND, G]
        for j in range(G):
            out[order[j * NC + c], :] = o[:, j]
    return out
